# revision 22
# baseline (speedup 1.0000x reference)
"""BotRGCN on 8 TRN2 NeuronCores (Bass/Tile SPMD kernel), v2.

Design (graph/data parallel; nodes sharded, weights replicated):
  - Activations live TRANSPOSED on-chip: xT [feat(128 part), nodes(free)] fp16.
    Every matmul contracts the partition dim; there are ZERO transposes.
  - Per RGCN layer, each core projects its node shard with both relation
    weights at once (rhs = [Wr0|Wr1], one matmul per 128-node window) giving
    natural-layout rows T = [x@W0 | x@W1] [npc, 256] fp16, which are
    all-gathered into a full table [N, 256] in HBM (addr_space=Shared).
  - Edges are grouped by (dst-window, relation, src-quarter); each group gets
    ceil(max-core-count/128) chunk slots of 128 edges (per-group, so padding
    stays ~35% instead of a uniform worst case). dma_gather pulls T rows
    (elem 128 fp16, row stride 256) for each slot, round-robining the 4 SWDGE
    queues so descriptor generation runs 4-wide on the gpsimd Q7 cores.
  - Scatter-add is a one-hot matmul: psum_win[f, dst] += stag.T @ M where
    M[e, dst&127] = 1/cnt(dst,rel) (mean weights folded in, built on DVE as
    one fp16 tensor_scalar is_eq*mult against an iota row).
  - The Wroot term joins the same psum (lhsT=Wroot, rhs=xT[:, win]); brgcn
    rides the psum->SBUF copy on the Scalar engine (Identity + bias AP).
  - The output head (leaky(x@Wo+bo)@Wout+bout) is fused into layer 2's
    per-window loop; leaky-relus use the Scalar engine's Lrelu activation.
"""

import math
from contextlib import ExitStack

import numpy as np

import concourse.bacc as bacc
import concourse.bass as bass
import concourse.mybir as mybir
import concourse.tile as tile
from concourse import bass_utils

F32 = mybir.dt.float32
F16 = mybir.dt.float16
I16 = mybir.dt.int16
AF = mybir.ActivationFunctionType
SLOPE = 0.01
import os

N_CORES = 8
NQ = 4          # src quarters (int16 gather-index range)
WB = int(os.environ.get("K_WB", "8"))
NSTAG = int(os.environ.get("K_NSTAG", "20"))
NQUEUES = int(os.environ.get("K_NQUEUES", "4"))
NOGATHER = os.environ.get("K_NOGATHER", "0") == "1"
NOCOLL = os.environ.get("K_NOCOLL", "0") == "1"
NOMASK = os.environ.get("K_NOMASK", "0") == "1"
SCRATCH = int(os.environ.get("K_SCRATCH", "16384"))
SHARED = os.environ.get("K_SHARED", "1") == "1"


# ---------------------------------------------------------------------------
# Host-side preprocessing
# ---------------------------------------------------------------------------

def _preprocess(edge_index, edge_type, n_nodes, n_cores=N_CORES):
    src = np.asarray(edge_index[0], dtype=np.int64)
    dst = np.asarray(edge_index[1], dtype=np.int64)
    et = np.asarray(edge_type, dtype=np.int64)
    E = src.shape[0]
    npc = n_nodes // n_cores
    assert npc * n_cores == n_nodes
    nw = (npc + 127) // 128
    qrows = (n_nodes + NQ - 1) // NQ
    assert qrows - 1 <= np.iinfo(np.int16).max

    # mean weights 1/cnt(dst, rel)
    seg = dst * 2 + et
    cnt = np.bincount(seg, minlength=2 * n_nodes)
    w_edge = (1.0 / np.maximum(cnt[seg], 1)).astype(np.float32)

    assert npc % NQ == 0
    npcq = npc // NQ
    core = dst // npc
    dstl = dst - core * npc
    win = dstl >> 7
    key = (dstl & 127).astype(np.float32)
    # table rows are gathered per sub-AllGather block: node (c, l) lands at
    # row k*qrows + c*npcq + (l - k*npcq) where k = l // npcq
    score = src // npc
    sloc = src - score * npc
    q = sloc // npcq
    idxv = (score * npcq + (sloc - q * npcq)).astype(np.int16)

    ng = nw * 2 * NQ                       # groups per core
    gid = (win * 2 + et) * NQ + q          # per-core group id
    cntm = np.zeros((n_cores, ng), np.int64)
    np.add.at(cntm, (core, gid), 1)
    kq = np.maximum(1, -(-cntm.max(axis=0) // 128))   # [ng] chunk slots

    # call order: (batch, rel, quarter); slots within call: (win, k)
    # calls are split at MAX_CALL_SLOTS (SWDGE ring limit ~1024 descriptors)
    MAX_CALL_SLOTS = int(os.environ.get("K_MAXCS", "8"))
    nb = (nw + WB - 1) // WB
    slot_base = np.zeros(ng, np.int64)
    batch_calls = [[] for _ in range(nb)]   # per batch: (r, q, base, n_slots)
    S = 0
    for b in range(nb):
        wins = range(b * WB, min(nw, (b + 1) * WB))
        for r in range(2):
            for qq in range(NQ):
                base = S
                for w_ in wins:
                    g = (w_ * 2 + r) * NQ + qq
                    slot_base[g] = S
                    S += int(kq[g])
                for c0 in range(base, S, MAX_CALL_SLOTS):
                    batch_calls[b].append(
                        (r, qq, c0, min(MAX_CALL_SLOTS, S - c0)))

    # place edges
    order = np.argsort(core * ng + gid, kind="stable")
    gidc_s = (core * ng + gid)[order]
    counts = np.bincount(gidc_s, minlength=n_cores * ng)
    starts = np.zeros(counts.size + 1, np.int64)
    np.cumsum(counts, out=starts[1:])
    pos = np.arange(E, dtype=np.int64) - starts[gidc_s]

    gid_s = gid[order]
    core_s = core[order]
    e_slot = slot_base[gid_s] + (pos >> 7)
    e_p = pos & 127

    gidx = np.zeros((n_cores, S * 128), np.int16)
    keym = np.full((n_cores, 128, S), -1.0, np.float32)
    wgtm = np.zeros((n_cores, 128, S), np.float32)
    gidx[core_s, e_slot * 128 + e_p] = idxv[order]
    keym[core_s, e_p, e_slot] = key[order]
    wgtm[core_s, e_p, e_slot] = w_edge[order]

    # wrapped idx layout: flat position i -> [i%16, i//16], replicated to 128
    idx16 = np.ascontiguousarray(
        gidx.reshape(n_cores, S * 8, 16).transpose(0, 2, 1))
    idx16 = np.tile(idx16, (1, 8, 1))       # [n_cores, 128, S*8]

    return dict(npc=npc, nw=nw, nb=nb, qrows=qrows, kq=kq,
                slot_base=slot_base, batch_calls=batch_calls, S=S,
                idx16=idx16, keym=keym.astype(np.float16),
                wgtm=wgtm.astype(np.float16))


# ---------------------------------------------------------------------------
# Device module
# ---------------------------------------------------------------------------

def _build_module(N, T, prep, n_cores=N_CORES, single_core=False):
    D = 128
    KT = T // 128
    npc = prep["npc"]
    nw = prep["nw"]
    nb = prep["nb"]
    qrows = prep["qrows"]
    kq = prep["kq"]
    slot_base = prep["slot_base"]
    batch_calls = prep["batch_calls"]
    S = prep["S"]
    npad = nw * 128
    TILE_W = 512
    NT = (npc + TILE_W - 1) // TILE_W
    MAXCS = max(c[3] for bc in batch_calls for c in bc)
    max_call_slots = MAXCS
    BSL_MAX = max(sum(c[3] for c in bc) for bc in batch_calls)
    assert KT * 128 == T

    if single_core:
        n_cores = 1
    nc = bacc.Bacc("TRN2", target_bir_lowering=False, debug=False,
                   enable_asserts=False, num_devices=n_cores,
                   num_swdge_queues=NQUEUES,
                   dynamic_dma_scratch_size=SCRATCH)

    # ---- I/O -------------------------------------------------------------
    tweetT_d = nc.dram_tensor("tweetT", [128, NT * KT * TILE_W], F16,
                              kind="ExternalInput")
    idx_d = nc.dram_tensor("idx16", [128, S * 8], I16, kind="ExternalInput")
    keym_d = nc.dram_tensor("keym", [128, S], F16, kind="ExternalInput")
    wgtm_d = nc.dram_tensor("wgtm", [128, S], F16, kind="ExternalInput")
    wt_d = nc.dram_tensor("wt", [T, D], F16, kind="ExternalInput")
    wi_d = nc.dram_tensor("wi", [D, D], F16, kind="ExternalInput")
    wrcat_d = nc.dram_tensor("wrcat", [D, 2 * D], F16, kind="ExternalInput")
    wroot_d = nc.dram_tensor("wroot", [D, D], F16, kind="ExternalInput")
    wo_d = nc.dram_tensor("wo", [D, D], F16, kind="ExternalInput")
    wout_d = nc.dram_tensor("wout", [D, 2], F16, kind="ExternalInput")
    bt_d = nc.dram_tensor("bt", [D, 1], F32, kind="ExternalInput")
    bi_d = nc.dram_tensor("bi", [D, 1], F32, kind="ExternalInput")
    brgcn_d = nc.dram_tensor("brgcn", [D, 1], F32, kind="ExternalInput")
    bo_d = nc.dram_tensor("bo", [D, 1], F32, kind="ExternalInput")
    bout_d = nc.dram_tensor("bout", [2, 1], F32, kind="ExternalInput")
    out_d = nc.dram_tensor("out", [npc, 2], F32, kind="ExternalOutput")
    tab_in = {}
    if single_core:
        tab_in["t1"] = nc.dram_tensor("table1_in", [N, 256], F16,
                                      kind="ExternalInput")
        tab_in["t2"] = nc.dram_tensor("table2_in", [N, 256], F16,
                                      kind="ExternalInput")

    rg = [list(range(n_cores))]

    with tile.TileContext(nc) as tc, ExitStack() as ctx:
        wpool = ctx.enter_context(tc.tile_pool(name="wpool", bufs=1))
        wt_sb = wpool.tile([128, KT * 128], F16)
        for k in range(KT):
            nc.sync.dma_start(out=wt_sb[:, k * 128:(k + 1) * 128],
                              in_=wt_d[k * 128:(k + 1) * 128, :])
        wi_sb = wpool.tile([128, 128], F16)
        nc.sync.dma_start(out=wi_sb[:], in_=wi_d[:, :])
        wrcat_sb = wpool.tile([128, 256], F16)
        nc.sync.dma_start(out=wrcat_sb[:], in_=wrcat_d[:, :])
        wroot_sb = wpool.tile([128, 128], F16)
        nc.sync.dma_start(out=wroot_sb[:], in_=wroot_d[:, :])
        wo_sb = wpool.tile([128, 128], F16)
        nc.sync.dma_start(out=wo_sb[:], in_=wo_d[:, :])
        wout_sb = wpool.tile([128, 2], F16)
        nc.sync.dma_start(out=wout_sb[:], in_=wout_d[:, :])
        bt_sb = wpool.tile([128, 1], F32)
        nc.sync.dma_start(out=bt_sb[:], in_=bt_d[:, :])
        bi_sb = wpool.tile([128, 1], F32)
        nc.sync.dma_start(out=bi_sb[:], in_=bi_d[:, :])
        brgcn_sb = wpool.tile([128, 1], F32)
        nc.sync.dma_start(out=brgcn_sb[:], in_=brgcn_d[:, :])
        bo_sb = wpool.tile([128, 1], F32)
        nc.sync.dma_start(out=bo_sb[:], in_=bo_d[:, :])
        bout_sb = wpool.tile([2, 1], F32)
        nc.sync.dma_start(out=bout_sb[:], in_=bout_d[:, :])

        idx_sb = wpool.tile([128, S * 8], I16)
        nc.sync.dma_start(out=idx_sb[:], in_=idx_d[:, :])
        keym_sb = wpool.tile([128, S], F16)
        nc.sync.dma_start(out=keym_sb[:], in_=keym_d[:, :])
        wgtm_sb = wpool.tile([128, S], F16)
        nc.sync.dma_start(out=wgtm_sb[:], in_=wgtm_d[:, :])

        iota_sb = wpool.tile([128, 128], F16)
        nc.gpsimd.iota(iota_sb[:], pattern=[[1, 128]], base=0,
                       channel_multiplier=0,
                       allow_small_or_imprecise_dtypes=True)

        # persistent transposed activations (fp16)
        xa = wpool.tile([128, npad], F16)
        xb = wpool.tile([128, npad], F16)
        obuf = wpool.tile([128, 2 * nw], F32)

        stag = [wpool.tile([128, max_call_slots * 128], F16, name=f"st{i}")
                for i in range(NSTAG)]

        dpool = ctx.enter_context(tc.tile_pool(name="dpool", bufs=1,
                                               space="DRAM"))
        npcq_l = npc // NQ
        ag1_in = dpool.tile([npc, 256], F16)
        ag2_in = dpool.tile([npc, 256], F16)
        qrows_l = prep["qrows"]
        if single_core:
            table1 = [tab_in["t1"][k * qrows_l:(k + 1) * qrows_l, :]
                      for k in range(NQ)]
            table2 = [tab_in["t2"][k * qrows_l:(k + 1) * qrows_l, :]
                      for k in range(NQ)]
        else:
            aspace = "Shared" if SHARED else "Local"
            table1 = [dpool.tile([qrows_l, 256], F16, addr_space=aspace,
                                 name=f"tab1_{k}") for k in range(NQ)]
            table2 = [dpool.tile([qrows_l, 256], F16, addr_space=aspace,
                                 name=f"tab2_{k}") for k in range(NQ)]

        def project_win(xsrc, w, proj_ps, proj_sb, agq):
            # T rows for window w: [wsz, 256] = x_win @ [Wr0|Wr1]
            wsz = min(128, npc - w * 128)
            ps = proj_ps.tile([128, 256], F32, name="psT")
            nc.tensor.matmul(ps[:wsz, :], lhsT=xsrc[:, w * 128:w * 128 + wsz],
                             rhs=wrcat_sb[:], start=True, stop=True)
            tn = proj_sb.tile([128, 256], F16, name="tn")
            nc.scalar.activation(out=tn[:wsz, :], in_=ps[:wsz, :],
                                 func=AF.Identity)
            nc.scalar.dma_start(out=agq[w * 128:w * 128 + wsz, :],
                                in_=tn[:wsz, :])

        # ---- stage 1: x = leaky(leaky(tweet@Wt+bt)@Wi+bi), project, AG ---
        with tc.tile_pool(name="s1ps", bufs=2, space="PSUM") as s1ps, \
             tc.tile_pool(name="s1ps2", bufs=2, space="PSUM") as s1ps2, \
             tc.tile_pool(name="s1buf", bufs=4) as s1buf, \
             tc.tile_pool(name="prps", bufs=2, space="PSUM") as prps, \
             tc.tile_pool(name="prsb", bufs=3) as prsb:
            for t in range(NT):
                c0 = t * TILE_W
                cw = min(TILE_W, npc - c0)
                ps1 = s1ps.tile([128, TILE_W], F32, name="ps1")
                tw = s1buf.tile([128, KT * TILE_W], F16, name="tw")
                nc.sync.dma_start(
                    out=tw[:],
                    in_=tweetT_d[:, t * KT * TILE_W:(t + 1) * KT * TILE_W])
                for k in range(KT):
                    nc.tensor.matmul(ps1[:, :cw],
                                     lhsT=wt_sb[:, k * 128:(k + 1) * 128],
                                     rhs=tw[:, k * TILE_W:k * TILE_W + cw],
                                     start=(k == 0), stop=(k == KT - 1))
                x1 = s1buf.tile([128, TILE_W], F16, name="x1")
                nc.scalar.activation(out=x1[:, :cw], in_=ps1[:, :cw],
                                     func=AF.Lrelu, bias=bt_sb[:, :1],
                                     alpha=SLOPE)
                ps2 = s1ps2.tile([128, TILE_W], F32, name="ps2")
                nc.tensor.matmul(ps2[:, :cw], lhsT=wi_sb[:], rhs=x1[:, :cw],
                                 start=True, stop=True)
                nc.scalar.activation(out=xa[:, c0:c0 + cw], in_=ps2[:, :cw],
                                     func=AF.Lrelu, bias=bi_sb[:, :1],
                                     alpha=SLOPE)
                for w in range(c0 // 128, (c0 + cw + 127) // 128):
                    project_win(xa, w, prps, prsb, ag1_in)

        npcq = npc // NQ

        def all_gather(ag_in, table):
            if NOCOLL:
                return  # timing probe: tables left unwritten (garbage data)
            for k in range(NQ):
                nc.gpsimd.collective_compute(
                    "AllGather", mybir.AluOpType.bypass, replica_groups=rg,
                    ins=[ag_in[k * npcq:(k + 1) * npcq, :]],
                    outs=[table[k][:, :]])

        if not single_core:
            all_gather(ag1_in, table1)

        # ---- RGCN layer --------------------------------------------------
        def rgcn_layer(xin, xout, table, ag_next):
            head = ag_next is None
            with ExitStack() as lctx:
                aggp = lctx.enter_context(
                    tc.tile_pool(name="aggp", bufs=4 if head else 6,
                                 space="PSUM"))
                mp = lctx.enter_context(tc.tile_pool(name="mp", bufs=2))
                if head:
                    hps = lctx.enter_context(
                        tc.tile_pool(name="hps", bufs=2, space="PSUM"))
                    hps2 = lctx.enter_context(
                        tc.tile_pool(name="hps2", bufs=2, space="PSUM"))
                    hbuf = lctx.enter_context(
                        tc.tile_pool(name="hbuf", bufs=4))
                else:
                    prps = lctx.enter_context(
                        tc.tile_pool(name="prps", bufs=2, space="PSUM"))
                    prsb = lctx.enter_context(
                        tc.tile_pool(name="prsb", bufs=3))
                ci = 0
                for b in range(nb):
                    wins = list(range(b * WB, min(nw, (b + 1) * WB)))
                    slot2st = {}
                    calls = batch_calls[b]
                    B0 = calls[0][2]
                    BSL = sum(c[3] for c in calls)
                    for (r, qq, base, nsl) in calls:
                        st = stag[ci % NSTAG]
                        n_i = nsl * 128
                        if not NOGATHER:
                            nc.gpsimd.dma_gather(
                                out_ap=st[:, :nsl * 128].rearrange(
                                    "p (c d) -> p c d", d=128),
                                in_ap=table[qq][:, r * 128:(r + 1) * 128],
                                idxs_ap=idx_sb[:, base * 8:
                                               (base + nsl) * 8],
                                num_idxs=n_i, num_idxs_reg=n_i,
                                elem_size=128, elem_step=256,
                                queue_num=ci % NQUEUES)
                        for sl in range(base, base + nsl):
                            slot2st[sl] = (st, sl - base)
                        ci += 1
                    # one weighted one-hot build for the whole batch:
                    # m[p, s, d] = (iota[p, d] == keym[p, B0+s])
                    #              * wgtm[p, B0+s]
                    mcb = mp.tile([128, BSL_MAX * 128], F16, name="mcb")
                    i_ap = iota_sb[:]
                    i_b = bass.AP(i_ap.tensor, i_ap.offset,
                                  [i_ap.ap[0], [0, BSL], [1, 128]])
                    k_ap = keym_sb[:, B0:B0 + BSL]
                    k_b = bass.AP(k_ap.tensor, k_ap.offset,
                                  [k_ap.ap[0], [1, BSL], [0, 128]])
                    w_ap = wgtm_sb[:, B0:B0 + BSL]
                    w_b = bass.AP(w_ap.tensor, w_ap.offset,
                                  [w_ap.ap[0], [1, BSL], [0, 128]])
                    mv = mcb[:, :BSL * 128].rearrange(
                        "p (s d) -> p s d", d=128)
                    if not NOMASK:
                        nc.vector.tensor_tensor(
                            out=mv, in0=i_b, in1=k_b,
                            op=mybir.AluOpType.is_equal)
                        nc.vector.tensor_tensor(
                            out=mv, in0=mv, in1=w_b,
                            op=mybir.AluOpType.mult)
                    for w in wins:
                        wsz = min(128, npc - w * 128)
                        pwt = aggp.tile([128, 128], F32, name="pw")
                        pw = pwt[:, :]
                        slots = []
                        for r in range(2):
                            for qq in range(NQ):
                                g = (w * 2 + r) * NQ + qq
                                for k in range(int(kq[g])):
                                    sl = int(slot_base[g]) + k
                                    st, off = slot2st[sl]
                                    slots.append((st, off, sl - B0))
                        for i, (st, off, moff) in enumerate(slots):
                            nc.tensor.matmul(
                                pw[:, :],
                                lhsT=st[:, off * 128:(off + 1) * 128],
                                rhs=mcb[:, moff * 128:(moff + 1) * 128],
                                start=(i == 0), stop=False)
                        nc.tensor.matmul(pw[:, :wsz], lhsT=wroot_sb[:],
                                         rhs=xin[:, w * 128:w * 128 + wsz],
                                         start=False, stop=True)
                        nc.scalar.activation(
                            out=xout[:, w * 128:w * 128 + wsz],
                            in_=pw[:, :wsz], func=AF.Identity,
                            bias=brgcn_sb[:, :1])
                        if not head:
                            project_win(xout, w, prps, prsb, ag_next)
                        else:
                            pht = hps.tile([128, 128], F32, name="ph")
                            ph = pht[:, :]
                            nc.tensor.matmul(
                                ph[:, :wsz], lhsT=wo_sb[:],
                                rhs=xout[:, w * 128:w * 128 + wsz],
                                start=True, stop=True)
                            h = hbuf.tile([128, 128], F16, name="h")
                            nc.scalar.activation(out=h[:, :wsz],
                                                 in_=ph[:, :wsz],
                                                 func=AF.Lrelu,
                                                 bias=bo_sb[:, :1],
                                                 alpha=SLOPE)
                            # natural rows: po[dst, c] = h.T @ Wout
                            pot = hps2.tile([128, 2], F32, name="po")
                            nc.tensor.matmul(pot[:wsz, :], lhsT=h[:, :wsz],
                                             rhs=wout_sb[:],
                                             start=True, stop=True)
                            nc.scalar.activation(
                                out=obuf[:wsz, 2 * w:2 * w + 2],
                                in_=pot[:wsz, :], func=AF.Identity)

        rgcn_layer(xa, xb, table1, ag2_in)
        if not single_core:
            all_gather(ag2_in, table2)
        rgcn_layer(xb, xa, table2, None)

        # flush the head output: rows (w, p) -> out_d[w*128 + p, :]
        nwf = npc // 128
        o_ap = out_d[:, :]
        nc.sync.dma_start(
            out=bass.AP(o_ap.tensor, o_ap.offset,
                        [[2, 128], [256, nwf], [1, 2]]),
            in_=obuf[:, :2 * nwf])
        if npc % 128:
            wsz = npc - nwf * 128
            nc.sync.dma_start(out=out_d[nwf * 128:, :],
                              in_=obuf[:wsz, 2 * nwf:2 * nwf + 2])

    nc.compile()
    return nc


# ---------------------------------------------------------------------------
# Public entry point
# ---------------------------------------------------------------------------

_CACHE = {}


def _get_module(N, T, prep):
    key = (N, T, prep["npc"], prep["S"])
    if key not in _CACHE:
        _CACHE[key] = _build_module(N, T, prep)
    return _CACHE[key]


def _make_in_maps(tweet, prep, Wt, bt, Wi, bi, Wrel, Wroot, brgcn, Wo, bo,
                  Wout, bout, n_cores=N_CORES):
    npc = prep["npc"]
    f32 = np.float32
    f16 = np.float16
    shared = dict(
        wt=np.ascontiguousarray(np.asarray(Wt, f32).astype(f16)),
        wi=np.ascontiguousarray(np.asarray(Wi, f32).astype(f16)),
        wrcat=np.ascontiguousarray(np.concatenate(
            [np.asarray(Wrel[0], f32), np.asarray(Wrel[1], f32)],
            axis=1).astype(f16)),
        wroot=np.ascontiguousarray(np.asarray(Wroot, f32).astype(f16)),
        wo=np.ascontiguousarray(np.asarray(Wo, f32).astype(f16)),
        wout=np.ascontiguousarray(np.asarray(Wout, f32).astype(f16)),
        bt=np.ascontiguousarray(np.reshape(bt, (-1, 1)), f32),
        bi=np.ascontiguousarray(np.reshape(bi, (-1, 1)), f32),
        brgcn=np.ascontiguousarray(np.reshape(brgcn, (-1, 1)), f32),
        bo=np.ascontiguousarray(np.reshape(bo, (-1, 1)), f32),
        bout=np.ascontiguousarray(np.reshape(bout, (-1, 1)), f32),
    )
    in_maps = []
    for c in range(n_cores):
        m = dict(shared)
        tT = tweet[c * npc:(c + 1) * npc].T.astype(f16)   # [T, npc]
        KT = tT.shape[0] // 128
        TILE_W = 512
        NT = (npc + TILE_W - 1) // TILE_W
        buf = np.zeros((128, NT * KT * TILE_W), f16)
        for t in range(NT):
            cw = min(TILE_W, npc - t * TILE_W)
            for k in range(KT):
                blk = buf[:, (t * KT + k) * TILE_W:(t * KT + k) * TILE_W + cw]
                blk[:] = tT[k * 128:(k + 1) * 128,
                            t * TILE_W:t * TILE_W + cw]
        m["tweetT"] = np.ascontiguousarray(buf)
        m["idx16"] = np.ascontiguousarray(prep["idx16"][c])
        m["keym"] = np.ascontiguousarray(prep["keym"][c])
        m["wgtm"] = np.ascontiguousarray(prep["wgtm"][c])
        in_maps.append(m)
    return in_maps


def kernel(tweet, edge_index, edge_type, Wt, bt, Wi, bi, Wrel, Wroot, brgcn,
           Wo, bo, Wout, bout):
    tweet = np.asarray(tweet, dtype=np.float32)
    N, T = tweet.shape
    prep = _preprocess(edge_index, edge_type, N)
    nc = _get_module(N, T, prep)
    in_maps = _make_in_maps(tweet, prep, Wt, bt, Wi, bi, Wrel, Wroot, brgcn,
                            Wo, bo, Wout, bout)
    res = bass_utils.run_bass_kernel_spmd(
        nc, in_maps, core_ids=list(range(N_CORES)))
    out = np.concatenate(
        [res.results[c]["out"] for c in range(N_CORES)], axis=0)
    out = out + np.asarray(bout, np.float32)[None, :]
    return np.ascontiguousarray(out, dtype=np.float32)



# revision 29
# speedup vs baseline: 6.6371x; 6.6371x over previous
"""BotRGCN on 8 TRN2 NeuronCores (Bass/Tile SPMD kernel), v2.

Design (graph/data parallel; nodes sharded, weights replicated):
  - Activations live TRANSPOSED on-chip: xT [feat(128 part), nodes(free)] fp16.
    Every matmul contracts the partition dim; there are ZERO transposes.
  - Per RGCN layer, each core projects its node shard with both relation
    weights at once (rhs = [Wr0|Wr1], one matmul per 128-node window) giving
    natural-layout rows T = [x@W0 | x@W1] [npc, 256] fp16, which are
    all-gathered into a full table [N, 256] in HBM (addr_space=Shared).
  - Edges are grouped by (dst-window, relation, src-quarter); each group gets
    ceil(max-core-count/128) chunk slots of 128 edges (per-group, so padding
    stays ~35% instead of a uniform worst case). dma_gather pulls T rows
    (elem 128 fp16, row stride 256) for each slot, round-robining the 4 SWDGE
    queues so descriptor generation runs 4-wide on the gpsimd Q7 cores.
  - Scatter-add is a one-hot matmul: psum_win[f, dst] += stag.T @ M where
    M[e, dst&127] = 1/cnt(dst,rel) (mean weights folded in, built on DVE as
    one fp16 tensor_scalar is_eq*mult against an iota row).
  - The Wroot term joins the same psum (lhsT=Wroot, rhs=xT[:, win]); brgcn
    rides the psum->SBUF copy on the Scalar engine (Identity + bias AP).
  - The output head (leaky(x@Wo+bo)@Wout+bout) is fused into layer 2's
    per-window loop; leaky-relus use the Scalar engine's Lrelu activation.
"""

import math
from contextlib import ExitStack

import numpy as np

import concourse.bacc as bacc
import concourse.bass as bass
import concourse.mybir as mybir
import concourse.tile as tile
from concourse import bass_utils

F32 = mybir.dt.float32
F16 = mybir.dt.float16
F8 = mybir.dt.float8e4
I16 = mybir.dt.int16
AF = mybir.ActivationFunctionType
SLOPE = 0.01
import os

N_CORES = 8
NQ = 4          # src quarters (int16 gather-index range)
WB = int(os.environ.get("K_WB", "8"))
NSTAG = int(os.environ.get("K_NSTAG", "20"))
NQUEUES = int(os.environ.get("K_NQUEUES", "4"))
NOGATHER = os.environ.get("K_NOGATHER", "0") == "1"
NOCOLL = os.environ.get("K_NOCOLL", "0") == "1"
NOMASK = os.environ.get("K_NOMASK", "0") == "1"
FP8TAB = os.environ.get("K_FP8", "0") == "1"
SCRATCH = int(os.environ.get("K_SCRATCH", "16384"))
SHARED = os.environ.get("K_SHARED", "1") == "1"


# ---------------------------------------------------------------------------
# Host-side preprocessing
# ---------------------------------------------------------------------------

def _preprocess(edge_index, edge_type, n_nodes, n_cores=N_CORES):
    src = np.asarray(edge_index[0], dtype=np.int64)
    dst = np.asarray(edge_index[1], dtype=np.int64)
    et = np.asarray(edge_type, dtype=np.int64)
    E = src.shape[0]
    npc = n_nodes // n_cores
    assert npc * n_cores == n_nodes
    nw = (npc + 127) // 128
    qrows = (n_nodes + NQ - 1) // NQ
    assert qrows - 1 <= np.iinfo(np.int16).max

    # mean weights 1/cnt(dst, rel)
    seg = dst * 2 + et
    cnt = np.bincount(seg, minlength=2 * n_nodes)
    w_edge = (1.0 / np.maximum(cnt[seg], 1)).astype(np.float32)

    assert npc % NQ == 0
    npcq = npc // NQ
    core = dst // npc
    dstl = dst - core * npc
    win = dstl >> 7
    key = (dstl & 127).astype(np.float32)
    # table rows are gathered per sub-AllGather block: node (c, l) lands at
    # row k*qrows + c*npcq + (l - k*npcq) where k = l // npcq
    score = src // npc
    sloc = src - score * npc
    q = sloc // npcq
    idxv = (score * npcq + (sloc - q * npcq)).astype(np.int16)

    ng = nw * 2 * NQ                       # groups per core
    gid = (win * 2 + et) * NQ + q          # per-core group id
    cntm = np.zeros((n_cores, ng), np.int64)
    np.add.at(cntm, (core, gid), 1)
    kq = np.maximum(1, -(-cntm.max(axis=0) // 128))   # [ng] chunk slots

    # call order: (batch, rel, quarter); slots within call: (win, k)
    # calls are split at MAX_CALL_SLOTS (SWDGE ring limit ~1024 descriptors)
    MAX_CALL_SLOTS = int(os.environ.get("K_MAXCS", "8"))
    nb = (nw + WB - 1) // WB
    slot_base = np.zeros(ng, np.int64)
    batch_calls = [[] for _ in range(nb)]   # per batch: (r, q, base, n_slots)
    S = 0
    for b in range(nb):
        wins = range(b * WB, min(nw, (b + 1) * WB))
        for r in range(2):
            for qq in range(NQ):
                base = S
                for w_ in wins:
                    g = (w_ * 2 + r) * NQ + qq
                    slot_base[g] = S
                    S += int(kq[g])
                for c0 in range(base, S, MAX_CALL_SLOTS):
                    batch_calls[b].append(
                        (r, qq, c0, min(MAX_CALL_SLOTS, S - c0)))

    # place edges
    order = np.argsort(core * ng + gid, kind="stable")
    gidc_s = (core * ng + gid)[order]
    counts = np.bincount(gidc_s, minlength=n_cores * ng)
    starts = np.zeros(counts.size + 1, np.int64)
    np.cumsum(counts, out=starts[1:])
    pos = np.arange(E, dtype=np.int64) - starts[gidc_s]

    gid_s = gid[order]
    core_s = core[order]
    e_slot = slot_base[gid_s] + (pos >> 7)
    e_p = pos & 127

    gidx = np.zeros((n_cores, S * 128), np.int16)
    gidx[core_s, e_slot * 128 + e_p] = idxv[order]

    # host-built weighted one-hot masks: maskm[c, p, s*128 + (dst&127)] = w
    maskm = np.zeros((n_cores, 128, S * 128), np.float16)
    maskm[core_s, e_p, e_slot * 128 + key[order].astype(np.int64)] = \
        w_edge[order]

    # wrapped idx layout: flat position i -> [i%16, i//16], replicated to 128
    idx16 = np.ascontiguousarray(
        gidx.reshape(n_cores, S * 8, 16).transpose(0, 2, 1))
    idx16 = np.tile(idx16, (1, 8, 1))       # [n_cores, 128, S*8]

    return dict(npc=npc, nw=nw, nb=nb, qrows=qrows, kq=kq,
                slot_base=slot_base, batch_calls=batch_calls, S=S,
                idx16=idx16, maskm=maskm)


# ---------------------------------------------------------------------------
# Device module
# ---------------------------------------------------------------------------

def _build_module(N, T, prep, n_cores=N_CORES, single_core=False):
    D = 128
    KT = T // 128
    npc = prep["npc"]
    nw = prep["nw"]
    nb = prep["nb"]
    qrows = prep["qrows"]
    kq = prep["kq"]
    slot_base = prep["slot_base"]
    batch_calls = prep["batch_calls"]
    S = prep["S"]
    npad = nw * 128
    TILE_W = 512
    NT = (npc + TILE_W - 1) // TILE_W
    MAXCS = max(c[3] for bc in batch_calls for c in bc)
    max_call_slots = MAXCS
    BSL_MAX = max(sum(c[3] for c in bc) for bc in batch_calls)
    assert KT * 128 == T

    if single_core:
        n_cores = 1
    nc = bacc.Bacc("TRN2", target_bir_lowering=False, debug=False,
                   enable_asserts=False, num_devices=n_cores,
                   num_swdge_queues=NQUEUES,
                   dynamic_dma_scratch_size=SCRATCH)

    # ---- I/O -------------------------------------------------------------
    tweetT_d = nc.dram_tensor("tweetT", [128, NT * KT * TILE_W], F16,
                              kind="ExternalInput")
    idx_d = nc.dram_tensor("idx16", [128, S * 8], I16, kind="ExternalInput")
    maskm_d = nc.dram_tensor("maskm", [128, S * 128], F16,
                             kind="ExternalInput")
    wt_d = nc.dram_tensor("wt", [T, D], F16, kind="ExternalInput")
    wi_d = nc.dram_tensor("wi", [D, D], F16, kind="ExternalInput")
    wrcat_d = nc.dram_tensor("wrcat", [D, 2 * D], F16, kind="ExternalInput")
    wroot_d = nc.dram_tensor("wroot", [D, D], F16, kind="ExternalInput")
    wo_d = nc.dram_tensor("wo", [D, D], F16, kind="ExternalInput")
    wout_d = nc.dram_tensor("wout", [D, 2], F16, kind="ExternalInput")
    bt_d = nc.dram_tensor("bt", [D, 1], F32, kind="ExternalInput")
    bi_d = nc.dram_tensor("bi", [D, 1], F32, kind="ExternalInput")
    brgcn_d = nc.dram_tensor("brgcn", [D, 1], F32, kind="ExternalInput")
    bo_d = nc.dram_tensor("bo", [D, 1], F32, kind="ExternalInput")
    bout_d = nc.dram_tensor("bout", [2, 1], F32, kind="ExternalInput")
    out_d = nc.dram_tensor("out", [npc, 2], F32, kind="ExternalOutput")
    tab_in = {}
    if single_core:
        tab_in["t1"] = nc.dram_tensor("table1_in", [N, 256], F16,
                                      kind="ExternalInput")
        tab_in["t2"] = nc.dram_tensor("table2_in", [N, 256], F16,
                                      kind="ExternalInput")

    rg = [list(range(n_cores))]

    with tile.TileContext(nc) as tc, ExitStack() as ctx:
        wpool = ctx.enter_context(tc.tile_pool(name="wpool", bufs=1))
        wt_sb = wpool.tile([128, KT * 128], F16)
        for k in range(KT):
            nc.sync.dma_start(out=wt_sb[:, k * 128:(k + 1) * 128],
                              in_=wt_d[k * 128:(k + 1) * 128, :])
        wi_sb = wpool.tile([128, 128], F16)
        nc.sync.dma_start(out=wi_sb[:], in_=wi_d[:, :])
        wrcat_sb = wpool.tile([128, 256], F16)
        nc.sync.dma_start(out=wrcat_sb[:], in_=wrcat_d[:, :])
        wroot_sb = wpool.tile([128, 128], F16)
        nc.sync.dma_start(out=wroot_sb[:], in_=wroot_d[:, :])
        wo_sb = wpool.tile([128, 128], F16)
        nc.sync.dma_start(out=wo_sb[:], in_=wo_d[:, :])
        wout_sb = wpool.tile([128, 2], F16)
        nc.sync.dma_start(out=wout_sb[:], in_=wout_d[:, :])
        bt_sb = wpool.tile([128, 1], F32)
        nc.sync.dma_start(out=bt_sb[:], in_=bt_d[:, :])
        bi_sb = wpool.tile([128, 1], F32)
        nc.sync.dma_start(out=bi_sb[:], in_=bi_d[:, :])
        brgcn_sb = wpool.tile([128, 1], F32)
        nc.sync.dma_start(out=brgcn_sb[:], in_=brgcn_d[:, :])
        bo_sb = wpool.tile([128, 1], F32)
        nc.sync.dma_start(out=bo_sb[:], in_=bo_d[:, :])
        bout_sb = wpool.tile([2, 1], F32)
        nc.sync.dma_start(out=bout_sb[:], in_=bout_d[:, :])

        idx_sb = wpool.tile([128, S * 8], I16)
        nc.sync.dma_start(out=idx_sb[:], in_=idx_d[:, :])

        # persistent transposed activations (fp16)
        xa = wpool.tile([128, npad], F16)
        xb = wpool.tile([128, npad], F16)
        obuf = wpool.tile([128, 2 * nw], F32)

        stag = [wpool.tile([128, max_call_slots * 128], F16, name=f"st{i}")
                for i in range(NSTAG)]

        dpool = ctx.enter_context(tc.tile_pool(name="dpool", bufs=1,
                                               space="DRAM"))
        npcq_l = npc // NQ
        ag1_in = dpool.tile([npc, 256], F16)
        ag2_in = dpool.tile([npc, 256], F16)
        qrows_l = prep["qrows"]
        if single_core:
            table1 = [tab_in["t1"][k * qrows_l:(k + 1) * qrows_l, :]
                      for k in range(NQ)]
            table2 = [tab_in["t2"][k * qrows_l:(k + 1) * qrows_l, :]
                      for k in range(NQ)]
        else:
            aspace = "Shared" if SHARED else "Local"
            table1 = [dpool.tile([qrows_l, 256], F16, addr_space=aspace,
                                 name=f"tab1_{k}") for k in range(NQ)]
            table2 = [dpool.tile([qrows_l, 256], F16, addr_space=aspace,
                                 name=f"tab2_{k}") for k in range(NQ)]

        def project_win(xsrc, w, proj_ps, proj_sb, agq):
            # T rows for window w: [wsz, 256] = x_win @ [Wr0|Wr1]
            wsz = min(128, npc - w * 128)
            ps = proj_ps.tile([128, 256], F32, name="psT")
            nc.tensor.matmul(ps[:wsz, :], lhsT=xsrc[:, w * 128:w * 128 + wsz],
                             rhs=wrcat_sb[:], start=True, stop=True)
            tn = proj_sb.tile([128, 256], F16, name="tn")
            nc.scalar.activation(out=tn[:wsz, :], in_=ps[:wsz, :],
                                 func=AF.Identity)
            nc.scalar.dma_start(out=agq[w * 128:w * 128 + wsz, :],
                                in_=tn[:wsz, :])

        # ---- stage 1: x = leaky(leaky(tweet@Wt+bt)@Wi+bi), project, AG ---
        with tc.tile_pool(name="s1ps", bufs=2, space="PSUM") as s1ps, \
             tc.tile_pool(name="s1ps2", bufs=2, space="PSUM") as s1ps2, \
             tc.tile_pool(name="s1buf", bufs=4) as s1buf, \
             tc.tile_pool(name="prps", bufs=2, space="PSUM") as prps, \
             tc.tile_pool(name="prsb", bufs=3) as prsb:
            for t in range(NT):
                c0 = t * TILE_W
                cw = min(TILE_W, npc - c0)
                ps1 = s1ps.tile([128, TILE_W], F32, name="ps1")
                tw = s1buf.tile([128, KT * TILE_W], F16, name="tw")
                nc.sync.dma_start(
                    out=tw[:],
                    in_=tweetT_d[:, t * KT * TILE_W:(t + 1) * KT * TILE_W])
                for k in range(KT):
                    nc.tensor.matmul(ps1[:, :cw],
                                     lhsT=wt_sb[:, k * 128:(k + 1) * 128],
                                     rhs=tw[:, k * TILE_W:k * TILE_W + cw],
                                     start=(k == 0), stop=(k == KT - 1))
                x1 = s1buf.tile([128, TILE_W], F16, name="x1")
                nc.scalar.activation(out=x1[:, :cw], in_=ps1[:, :cw],
                                     func=AF.Lrelu, bias=bt_sb[:, :1],
                                     alpha=SLOPE)
                ps2 = s1ps2.tile([128, TILE_W], F32, name="ps2")
                nc.tensor.matmul(ps2[:, :cw], lhsT=wi_sb[:], rhs=x1[:, :cw],
                                 start=True, stop=True)
                nc.scalar.activation(out=xa[:, c0:c0 + cw], in_=ps2[:, :cw],
                                     func=AF.Lrelu, bias=bi_sb[:, :1],
                                     alpha=SLOPE)
                for w in range(c0 // 128, (c0 + cw + 127) // 128):
                    project_win(xa, w, prps, prsb, ag1_in)

        npcq = npc // NQ

        def all_gather(ag_in, table):
            if NOCOLL:
                return  # timing probe: tables left unwritten (garbage data)
            for k in range(NQ):
                nc.gpsimd.collective_compute(
                    "AllGather", mybir.AluOpType.bypass, replica_groups=rg,
                    ins=[ag_in[k * npcq:(k + 1) * npcq, :]],
                    outs=[table[k][:, :]])

        if not single_core:
            all_gather(ag1_in, table1)

        # ---- RGCN layer --------------------------------------------------
        def rgcn_layer(xin, xout, table, ag_next):
            head = ag_next is None
            with ExitStack() as lctx:
                aggp = lctx.enter_context(
                    tc.tile_pool(name="aggp", bufs=4 if head else 6,
                                 space="PSUM"))
                mp = lctx.enter_context(tc.tile_pool(name="mp", bufs=2))
                if head:
                    hps = lctx.enter_context(
                        tc.tile_pool(name="hps", bufs=2, space="PSUM"))
                    hps2 = lctx.enter_context(
                        tc.tile_pool(name="hps2", bufs=2, space="PSUM"))
                    hbuf = lctx.enter_context(
                        tc.tile_pool(name="hbuf", bufs=4))
                else:
                    prps = lctx.enter_context(
                        tc.tile_pool(name="prps", bufs=2, space="PSUM"))
                    prsb = lctx.enter_context(
                        tc.tile_pool(name="prsb", bufs=3))
                ci = 0
                for b in range(nb):
                    wins = list(range(b * WB, min(nw, (b + 1) * WB)))
                    slot2st = {}
                    calls = batch_calls[b]
                    B0 = calls[0][2]
                    BSL = sum(c[3] for c in calls)
                    for (r, qq, base, nsl) in calls:
                        st = stag[ci % NSTAG]
                        n_i = nsl * 128
                        if not NOGATHER:
                            nc.gpsimd.dma_gather(
                                out_ap=st[:, :nsl * 128].rearrange(
                                    "p (c d) -> p c d", d=128),
                                in_ap=table[qq][:, r * 128:(r + 1) * 128],
                                idxs_ap=idx_sb[:, base * 8:
                                               (base + nsl) * 8],
                                num_idxs=n_i, num_idxs_reg=n_i,
                                elem_size=128, elem_step=256,
                                queue_num=ci % NQUEUES)
                        for sl in range(base, base + nsl):
                            slot2st[sl] = (st, sl - base)
                        ci += 1
                    # host-precomputed weighted one-hot masks, streamed in
                    mcb = mp.tile([128, BSL_MAX * 128], F16, name="mcb")
                    if not NOMASK:
                        nc.sync.dma_start(
                            out=mcb[:, :BSL * 128],
                            in_=maskm_d[:, B0 * 128:(B0 + BSL) * 128])
                    for w in wins:
                        wsz = min(128, npc - w * 128)
                        pwt = aggp.tile([128, 128], F32, name="pw")
                        pw = pwt[:, :]
                        slots = []
                        for r in range(2):
                            for qq in range(NQ):
                                g = (w * 2 + r) * NQ + qq
                                for k in range(int(kq[g])):
                                    sl = int(slot_base[g]) + k
                                    st, off = slot2st[sl]
                                    slots.append((st, off, sl - B0))
                        for i, (st, off, moff) in enumerate(slots):
                            nc.tensor.matmul(
                                pw[:, :],
                                lhsT=st[:, off * 128:(off + 1) * 128],
                                rhs=mcb[:, moff * 128:(moff + 1) * 128],
                                start=(i == 0), stop=False)
                        nc.tensor.matmul(pw[:, :wsz], lhsT=wroot_sb[:],
                                         rhs=xin[:, w * 128:w * 128 + wsz],
                                         start=False, stop=True)
                        nc.scalar.activation(
                            out=xout[:, w * 128:w * 128 + wsz],
                            in_=pw[:, :wsz], func=AF.Identity,
                            bias=brgcn_sb[:, :1])
                        if not head:
                            project_win(xout, w, prps, prsb, ag_next)
                        else:
                            pht = hps.tile([128, 128], F32, name="ph")
                            ph = pht[:, :]
                            nc.tensor.matmul(
                                ph[:, :wsz], lhsT=wo_sb[:],
                                rhs=xout[:, w * 128:w * 128 + wsz],
                                start=True, stop=True)
                            h = hbuf.tile([128, 128], F16, name="h")
                            nc.scalar.activation(out=h[:, :wsz],
                                                 in_=ph[:, :wsz],
                                                 func=AF.Lrelu,
                                                 bias=bo_sb[:, :1],
                                                 alpha=SLOPE)
                            # natural rows: po[dst, c] = h.T @ Wout
                            pot = hps2.tile([128, 2], F32, name="po")
                            nc.tensor.matmul(pot[:wsz, :], lhsT=h[:, :wsz],
                                             rhs=wout_sb[:],
                                             start=True, stop=True)
                            nc.scalar.activation(
                                out=obuf[:wsz, 2 * w:2 * w + 2],
                                in_=pot[:wsz, :], func=AF.Identity)

        rgcn_layer(xa, xb, table1, ag2_in)
        if not single_core:
            all_gather(ag2_in, table2)
        rgcn_layer(xb, xa, table2, None)

        # flush the head output: rows (w, p) -> out_d[w*128 + p, :]
        nwf = npc // 128
        o_ap = out_d[:, :]
        nc.sync.dma_start(
            out=bass.AP(o_ap.tensor, o_ap.offset,
                        [[2, 128], [256, nwf], [1, 2]]),
            in_=obuf[:, :2 * nwf])
        if npc % 128:
            wsz = npc - nwf * 128
            nc.sync.dma_start(out=out_d[nwf * 128:, :],
                              in_=obuf[:wsz, 2 * nwf:2 * nwf + 2])

    nc.compile()
    return nc


# ---------------------------------------------------------------------------
# Public entry point
# ---------------------------------------------------------------------------

_CACHE = {}


def _get_module(N, T, prep):
    key = (N, T, prep["npc"], prep["S"])
    if key not in _CACHE:
        _CACHE[key] = _build_module(N, T, prep)
    return _CACHE[key]


def _make_in_maps(tweet, prep, Wt, bt, Wi, bi, Wrel, Wroot, brgcn, Wo, bo,
                  Wout, bout, n_cores=N_CORES):
    npc = prep["npc"]
    f32 = np.float32
    f16 = np.float16
    shared = dict(
        wt=np.ascontiguousarray(np.asarray(Wt, f32).astype(f16)),
        wi=np.ascontiguousarray(np.asarray(Wi, f32).astype(f16)),
        wrcat=np.ascontiguousarray(np.concatenate(
            [np.asarray(Wrel[0], f32), np.asarray(Wrel[1], f32)],
            axis=1).astype(f16)),
        wroot=np.ascontiguousarray(np.asarray(Wroot, f32).astype(f16)),
        wo=np.ascontiguousarray(np.asarray(Wo, f32).astype(f16)),
        wout=np.ascontiguousarray(np.asarray(Wout, f32).astype(f16)),
        bt=np.ascontiguousarray(np.reshape(bt, (-1, 1)), f32),
        bi=np.ascontiguousarray(np.reshape(bi, (-1, 1)), f32),
        brgcn=np.ascontiguousarray(np.reshape(brgcn, (-1, 1)), f32),
        bo=np.ascontiguousarray(np.reshape(bo, (-1, 1)), f32),
        bout=np.ascontiguousarray(np.reshape(bout, (-1, 1)), f32),
    )
    in_maps = []
    for c in range(n_cores):
        m = dict(shared)
        tT = tweet[c * npc:(c + 1) * npc].T.astype(f16)   # [T, npc]
        KT = tT.shape[0] // 128
        TILE_W = 512
        NT = (npc + TILE_W - 1) // TILE_W
        buf = np.zeros((128, NT * KT * TILE_W), f16)
        for t in range(NT):
            cw = min(TILE_W, npc - t * TILE_W)
            for k in range(KT):
                blk = buf[:, (t * KT + k) * TILE_W:(t * KT + k) * TILE_W + cw]
                blk[:] = tT[k * 128:(k + 1) * 128,
                            t * TILE_W:t * TILE_W + cw]
        m["tweetT"] = np.ascontiguousarray(buf)
        m["idx16"] = np.ascontiguousarray(prep["idx16"][c])
        m["maskm"] = np.ascontiguousarray(prep["maskm"][c])
        in_maps.append(m)
    return in_maps


def kernel(tweet, edge_index, edge_type, Wt, bt, Wi, bi, Wrel, Wroot, brgcn,
           Wo, bo, Wout, bout):
    tweet = np.asarray(tweet, dtype=np.float32)
    N, T = tweet.shape
    prep = _preprocess(edge_index, edge_type, N)
    nc = _get_module(N, T, prep)
    in_maps = _make_in_maps(tweet, prep, Wt, bt, Wi, bi, Wrel, Wroot, brgcn,
                            Wo, bo, Wout, bout)
    res = bass_utils.run_bass_kernel_spmd(
        nc, in_maps, core_ids=list(range(N_CORES)))
    out = np.concatenate(
        [res.results[c]["out"] for c in range(N_CORES)], axis=0)
    out = out + np.asarray(bout, np.float32)[None, :]
    return np.ascontiguousarray(out, dtype=np.float32)

